# revision 1
# baseline (speedup 1.0000x reference)
"""Trainium2 Bass kernel for nn_CPIPre (GNN message passing + conv/attention).

Strategy (8 NeuronCores, SPMD + collectives):
  - adjacency A [8192, 8192] row-sharded: core c owns rows [1024c, 1024(c+1)).
    Host ships A_blk^T (fp8 e4m3, partition-major [128, 64*1024]) per core; it
    is DMA'd once into SBUF (8MB) and stays resident for all 3 GNN layers
    (memory roofline = read A once at fp8).
  - Per GNN layer: every core computes hs = relu(xs@Wg^T+b) for its own rows
    (tensor engine, bias folded in via an ones-row), AllGathers hs (bf16,
    20KB/rank), then computes its row block of A@hs as 128 accumulating
    matmuls (hs tile stationary [128,10], A^T moving [128,512], 4-way
    column-tiled PE). xs stays fp32-resident per core.
  - The [1,2] result needs mean(xs3) only, so layer 3 skips the xs update and
    AllGathers per-core partial sums [10,1] instead.
  - Conv branch (11x11 conv on the [4096,10] word embedding "image") is
    reformulated as one K=111 matmul per 512-column tile against a stacked
    shifted-copies tensor S [111, 4096] built by 11 shift-DMAs; bias via an
    ones-row in S. Attention + output MLP are tiny K<=21 matmuls.
  - Everything except the A row-block and index gathers is replicated; the
    final output is read from core 0.
"""
import numpy as np
import ml_dtypes

N = 8192
D = 10
NCORES = 8
NB = N // NCORES          # 1024 rows per core
NT = N // 128             # 64 contraction tiles
L = 4096
KW = 11
PAD = 5
LW = L + 2 * PAD          # 4106 padded width
LG = LC = LO = 3

BF16_NP = ml_dtypes.bfloat16
FP8_NP = ml_dtypes.float8_e4m3

_CACHE = {}


def _build_nc(reps=1, single_core=False):
    import concourse.bacc as bacc
    import concourse.mybir as mybir
    from concourse import tile

    F32 = mybir.dt.float32
    BF16 = mybir.dt.bfloat16
    FP8 = mybir.dt.float8e4
    AF = mybir.ActivationFunctionType
    ALU = mybir.AluOpType
    AX = mybir.AxisListType

    ndev = 1 if single_core else NCORES
    nc = bacc.Bacc("TRN2", target_bir_lowering=False, debug=False,
                   num_devices=ndev)

    at_d = nc.dram_tensor("AT", [128, NT * NB], FP8, kind="ExternalInput")
    xs0_d = nc.dram_tensor("XS0", [D + 1, NB], F32, kind="ExternalInput")
    s0_d = nc.dram_tensor("S0", [KW * D + 1, L], BF16, kind="ExternalInput")
    wg_d = nc.dram_tensor("WG", [LG, D + 1, D], F32, kind="ExternalInput")
    hst_d = nc.dram_tensor("HST", [LC, KW * D + 1, D], BF16, kind="ExternalInput")
    wa_d = nc.dram_tensor("WA", [D + 1, D], BF16, kind="ExternalInput")
    wo_d = nc.dram_tensor("WO", [LO, 2 * D + 1, 2 * D], F32, kind="ExternalInput")
    wi_d = nc.dram_tensor("WI", [2 * D + 1, 2], F32, kind="ExternalInput")
    sum8_d = nc.dram_tensor("SUM8", [NCORES * D, D], F32, kind="ExternalInput")
    ones_d = nc.dram_tensor("ONES", [1, LW], F32, kind="ExternalInput")
    onesb_d = nc.dram_tensor("ONESB", [1, LW], BF16, kind="ExternalInput")
    out_d = nc.dram_tensor("OUT", [2, 1], F32, kind="ExternalOutput")

    rg = [list(range(NCORES))]

    with tile.TileContext(nc) as tc:
        with (
            tc.tile_pool(name="const", bufs=1) as cp,
            tc.tile_pool(name="work", bufs=2) as wp,
            tc.tile_pool(name="pbig", bufs=2, space="PSUM") as pp_big,
            tc.tile_pool(name="psmall", bufs=2, space="PSUM") as pp_small,
            tc.tile_pool(name="pconv", bufs=2, space="PSUM") as pp_conv,
            tc.tile_pool(name="pattn", bufs=2, space="PSUM") as pp_attn,
            tc.tile_pool(name="dram", bufs=1, space="DRAM") as dp,
        ):
          for _rep in range(reps):
            at_sb = cp.tile([128, NT * NB], FP8)

            # ---------------- constants (emitted BEFORE the A load so their
            # DMAs drain first and PE can start smalls/conv/AG0 immediately) --
            wg_sb = cp.tile([D + 1, LG * D], F32)
            nc.sync.dma_start(wg_sb.rearrange("k (i d) -> k i d", d=D),
                              wg_d.rearrange("i k d -> k i d"))
            hst_sb = cp.tile([KW * D + 1, LC * D], BF16)
            nc.sync.dma_start(hst_sb.rearrange("k (i d) -> k i d", d=D),
                              hst_d.rearrange("i k d -> k i d"))
            wa_sb = cp.tile([D + 1, D], BF16)
            nc.sync.dma_start(wa_sb[:, :], wa_d[:, :])
            wo_sb = cp.tile([2 * D + 1, LO * 2 * D], F32)
            nc.sync.dma_start(wo_sb.rearrange("k (i d) -> k i d", d=2 * D),
                              wo_d.rearrange("i k d -> k i d"))
            wi_sb = cp.tile([2 * D + 1, 2], F32)
            nc.sync.dma_start(wi_sb[:, :], wi_d[:, :])
            sum8_sb = cp.tile([NCORES * D, D], F32)
            nc.sync.dma_start(sum8_sb[:, :], sum8_d[:, :])

            xs_cur = wp.tile([D + 1, NB], F32, name="xs0", tag="xs")
            nc.sync.dma_start(xs_cur[:, :], xs0_d[:, :])

            # conv buffers + S0 load (small, still ahead of the A bulk load)

            # conv buffers
            s_sb = cp.tile([KW * D + 1, L], BF16)
            ws1_sb = cp.tile([D, LW], BF16)
            ws2_sb = cp.tile([D, LW], BF16)
            ws3_sb = cp.tile([D + 1, LW], BF16)
            for w_sb in (ws1_sb, ws2_sb):
                nc.vector.memset(w_sb[:, 0:PAD], 0.0)
                nc.vector.memset(w_sb[:, PAD + L:LW], 0.0)
            nc.sync.dma_start(s_sb[:, :], s0_d[:, :])
            nc.sync.dma_start(ws3_sb[D:D + 1, PAD:PAD + L], onesb_d[0:1, 0:L])

            # ---------------- A^T resident bulk load (8 chunks) -------------
            for g in range(8):
                sl = slice(8 * NB * g, 8 * NB * (g + 1))
                nc.sync.dma_start(at_sb[:, sl], at_d[:, sl])

            # attention / MLP buffers
            hsa_sb = cp.tile([D, L], F32)
            wt_sb = cp.tile([1, L], F32)
            ys_part = cp.tile([D, 8], F32)
            part9 = cp.tile([D, 9], F32)
            ppart = cp.tile([D, 1], F32)
            ppro = cp.tile([D, 1], F32)
            hv = cp.tile([D, 1], F32)
            cvec = cp.tile([D + 1, 1], BF16)
            catv = cp.tile([2 * D + 1, LO + 1], F32)
            res_sb = cp.tile([2, 1], F32)
            ones10 = cp.tile([1, D], F32)
            nc.sync.dma_start(cvec[D:D + 1, :], onesb_d[0:1, 0:1])
            nc.sync.dma_start(catv[2 * D:2 * D + 1, :], ones_d[0:1, 0:LO + 1])
            nc.vector.memset(ones10[:, :], 1.0)

            ws_srcs = [None, ws1_sb, ws2_sb, ws3_sb]

            def conv_layer(i):
                src = ws_srcs[i]
                dst = ws_srcs[i + 1]
                if i > 0:
                    for s in range(KW):
                        nc.sync.dma_start(s_sb[D * s:D * (s + 1), 0:L],
                                          src[0:D, s:s + L])
                for half in range(2):
                    pc = pp_conv.tile([128, 512], F32,
                                      name=f"ps_c{i}_{half}", tag="conv")
                    for q in range(4):
                        nt = 4 * half + q
                        nc.tensor.matmul(
                            pc[32 * q:32 * q + D, :],
                            hst_sb[:, D * i:D * (i + 1)],
                            s_sb[:, 512 * nt:512 * (nt + 1)],
                            start=True, stop=True,
                            tile_position=(0, 32 * q),
                        )
                    for q in range(4):
                        nt = 4 * half + q
                        nc.scalar.activation(
                            dst[0:D, PAD + 512 * nt:PAD + 512 * (nt + 1)],
                            pc[32 * q:32 * q + D, :], AF.Relu)

            # ---------------- GNN layers ------------------------------------
            for i in range(LG):
                # hs for my rows: 8 matmuls [128,10] + relu->bf16
                ps_s = pp_small.tile([128, 8 * D], F32,
                                     name=f"ps_s{i}", tag="small")
                for t in range(8):
                    nc.tensor.matmul(
                        ps_s[:, D * t:D * (t + 1)],
                        xs_cur[:, 128 * t:128 * (t + 1)],
                        wg_sb[:, D * i:D * (i + 1)],
                        start=True, stop=True)
                hsl = wp.tile([128, 8 * D], BF16, name=f"hsl{i}", tag="hsl")
                nc.scalar.activation(hsl[:, :], ps_s[:, :], AF.Relu)

                cc_in = dp.tile([128, 8 * D], BF16,
                                name=f"cci{i}", tag=f"cci{i}")
                nc.sync.dma_start(cc_in[:, :], hsl[:, :])
                cc_out = dp.tile([128 * NCORES, 8 * D], BF16,
                                 name=f"cco{i}", tag=f"cco{i}",
                                 addr_space="Local" if single_core else "Shared")
                if single_core:
                    for r in range(NCORES):
                        nc.sync.dma_start(cc_out[128 * r:128 * (r + 1), :],
                                          cc_in[:, :])
                else:
                    nc.gpsimd.collective_compute(
                        "AllGather", ALU.bypass, replica_groups=rg,
                        ins=[cc_in.opt()], outs=[cc_out.opt()])
                hs_sb = wp.tile([128, NT * D], BF16, name=f"hs{i}", tag="hs")
                nc.sync.dma_start(
                    hs_sb.rearrange("p (r f) -> p r f", f=8 * D),
                    cc_out.rearrange("(r p) f -> p r f", p=128))

                # conv layer emitted here to interleave PE work
                if i < LC:
                    conv_layer(i)

                # big matmul: out^T [10, 1024] += hs_kt^T @ A^T_kt, col-tiled
                ps_b = []
                for h in range(2):
                    pb = pp_big.tile([128, 512], F32,
                                     name=f"ps_b{i}_{h}", tag="big")
                    for kt in range(NT):
                        g = kt % 4
                        nc.tensor.matmul(
                            pb[32 * g:32 * g + D, :],
                            hs_sb[:, D * kt:D * (kt + 1)],
                            at_sb[:, NB * kt + 512 * h:NB * kt + 512 * h + 512],
                            start=(kt < 4), stop=(kt >= NT - 4),
                            tile_position=(0, 32 * g),
                        )
                    ps_b.append(pb)

                if i < LG - 1:
                    xs_next = wp.tile([D + 1, NB], F32,
                                      name=f"xs{i + 1}", tag="xs")
                    nc.sync.dma_start(xs_next[D:D + 1, :], ones_d[0:1, 0:NB])
                    for h in range(2):
                        sl = slice(512 * h, 512 * (h + 1))
                        tmp = wp.tile([D, 512], F32,
                                      name=f"upd{i}_{h}", tag="upd")
                        nc.vector.scalar_tensor_tensor(
                            tmp[:, :], xs_cur[0:D, sl], 0.0,
                            ps_b[h][0:D, :], ALU.add, ALU.add)
                        nc.vector.tensor_add(tmp[:, :], tmp[:, :],
                                             ps_b[h][32:32 + D, :])
                        nc.vector.tensor_add(tmp[:, :], tmp[:, :],
                                             ps_b[h][64:64 + D, :])
                        nc.vector.tensor_add(xs_next[0:D, sl], tmp[:, :],
                                             ps_b[h][96:96 + D, :])
                    xs_cur = xs_next
                else:
                    # partial compound sums: sum over my rows of xs2 + A@hs2
                    nc.vector.reduce_sum(part9[:, 0:1], xs_cur[0:D, :],
                                         axis=AX.X)
                    for h in range(2):
                        for g in range(4):
                            col = 1 + 4 * h + g
                            nc.vector.reduce_sum(
                                part9[:, col:col + 1],
                                ps_b[h][32 * g:32 * g + D, :], axis=AX.X)
                    nc.vector.reduce_sum(ppart[:, :], part9[:, :], axis=AX.X)

            # hsa depends only on ws3 (conv L2) — own psum tag so it is not
            # serialized behind the cc2 collective by psum slot reuse
            for nt in range(8):
                pa = pp_conv.tile([128, 512], F32, name=f"ps_a{nt}", tag="conv")
                nc.tensor.matmul(pa[0:D, :], wa_sb[:, :],
                                 ws3_sb[:, PAD + 512 * nt:PAD + 512 * (nt + 1)],
                                 start=True, stop=True)
                nc.scalar.activation(hsa_sb[:, 512 * nt:512 * (nt + 1)],
                                     pa[0:D, :], AF.Relu)

            # ---------------- compound via tiny AllGather -------------------
            cc2_in = dp.tile([D, 1], F32, name="cc2i", tag="cc2i")
            nc.sync.dma_start(cc2_in[:, :], ppart[:, :])
            cc2_out = dp.tile([NCORES * D, 1], F32, name="cc2o", tag="cc2o",
                              addr_space="Local" if single_core else "Shared")
            if single_core:
                for r in range(NCORES):
                    nc.sync.dma_start(cc2_out[D * r:D * (r + 1), :],
                                      cc2_in[:, :])
            else:
                nc.gpsimd.collective_compute(
                    "AllGather", ALU.bypass, replica_groups=rg,
                    ins=[cc2_in.opt()], outs=[cc2_out.opt()])
            pc_sb = cp.tile([NCORES * D, 1], F32)
            nc.sync.dma_start(pc_sb[:, :], cc2_out[:, :])
            ps_cmp = pp_attn.tile([D, 1], F32, name="ps_cmp", tag="attn")
            nc.tensor.matmul(ps_cmp[:, :], sum8_sb[:, :], pc_sb[:, :],
                             start=True, stop=True)
            nc.scalar.activation(cvec[0:D, :], ps_cmp[:, :], AF.Copy,
                                 scale=1.0 / N)
            nc.scalar.activation(catv[0:D, 0:1], ps_cmp[:, :], AF.Copy,
                                 scale=1.0 / N)

            # ---------------- attention (post-AG part) ----------------------
            ph = pp_attn.tile([D, 1], F32, name="ps_h", tag="attn")
            nc.tensor.matmul(ph[:, :], wa_sb[:, :], cvec[:, :],
                             start=True, stop=True)
            nc.scalar.activation(hv[:, :], ph[:, :], AF.Relu)

            ys_scr = cp.tile([D, 512], F32)
            for nt in range(8):
                sl = slice(512 * nt, 512 * (nt + 1))
                pw = pp_attn.tile([1, 512], F32, name=f"ps_w{nt}", tag="attn")
                nc.tensor.matmul(pw[:, :], hv[:, :], hsa_sb[:, sl],
                                 start=True, stop=True)
                nc.scalar.activation(wt_sb[:, sl], pw[:, :], AF.Tanh)
                pbc = pp_attn.tile([D, 512], F32, name=f"ps_bc{nt}",
                                   tag="attn")
                nc.tensor.matmul(pbc[:, :], ones10[:, :], wt_sb[0:1, sl],
                                 start=True, stop=True)
                nc.vector.tensor_mul(ys_scr[:, :], hsa_sb[:, sl], pbc[:, :])
                nc.vector.reduce_sum(ys_part[:, nt:nt + 1], ys_scr[:, :],
                                     axis=AX.X)
            nc.vector.reduce_sum(ppro[:, :], ys_part[:, :], axis=AX.X)
            ppro2 = cp.tile([D, 1], F32)
            nc.scalar.activation(ppro2[:, :], ppro[:, :], AF.Copy,
                                 scale=1.0 / L)
            nc.sync.dma_start(catv[D:2 * D, 0:1], ppro2[:, :])

            # ---------------- output MLP ------------------------------------
            for j in range(LO):
                pm = pp_attn.tile([2 * D, 1], F32, name=f"ps_m{j}", tag="attn")
                nc.tensor.matmul(pm[:, :], wo_sb[:, 2 * D * j:2 * D * (j + 1)],
                                 catv[:, j:j + 1], start=True, stop=True)
                nc.scalar.activation(catv[0:2 * D, j + 1:j + 2], pm[:, :],
                                     AF.Relu)
            pf = pp_attn.tile([2, 1], F32, name="ps_f", tag="attn")
            nc.tensor.matmul(pf[:, :], wi_sb[:, :], catv[:, LO:LO + 1],
                             start=True, stop=True)
            nc.scalar.activation(res_sb[:, :], pf[:, :], AF.Copy)
            nc.sync.dma_start(out_d[:, :], res_sb[:, :])

    nc.compile()
    return nc


def _prep_inputs(inputs):
    fp = np.asarray(inputs["fingerprints"]).astype(np.int64)
    A = np.ascontiguousarray(np.asarray(inputs["adjacency"], dtype=np.float32))
    words = np.asarray(inputs["words"]).astype(np.int64)
    emb_fp = np.asarray(inputs["emb_fp"], dtype=np.float32)
    emb_word = np.asarray(inputs["emb_word"], dtype=np.float32)
    Wg_w = np.asarray(inputs["Wg_w"], dtype=np.float32)
    Wg_b = np.asarray(inputs["Wg_b"], dtype=np.float32)
    conv_w = np.asarray(inputs["conv_w"], dtype=np.float32)
    conv_b = np.asarray(inputs["conv_b"], dtype=np.float32)
    Wa_w = np.asarray(inputs["Wa_w"], dtype=np.float32)
    Wa_b = np.asarray(inputs["Wa_b"], dtype=np.float32)
    Wo_w = np.asarray(inputs["Wo_w"], dtype=np.float32)
    Wo_b = np.asarray(inputs["Wo_b"], dtype=np.float32)
    Wi_w = np.asarray(inputs["Wi_w"], dtype=np.float32)
    Wi_b = np.asarray(inputs["Wi_b"], dtype=np.float32)

    xs0 = emb_fp[fp]                                     # [N, D]
    ws0 = emb_word[words]                                # [L, D]

    A8 = A.astype(FP8_NP)
    shared = {}
    ws0_pad = np.zeros((D, LW), np.float32)
    ws0_pad[:, PAD:PAD + L] = ws0.T
    s0 = np.ones((KW * D + 1, L), np.float32)
    for s in range(KW):
        s0[D * s:D * (s + 1)] = ws0_pad[:, s:s + L]
    shared["S0"] = s0.astype(BF16_NP)

    shared["WG"] = np.stack(
        [np.concatenate([Wg_w[i].T, Wg_b[i][None, :]], 0) for i in range(LG)])

    hst = np.zeros((LC, KW * D + 1, D), np.float32)  # cast to bf16 below
    c_idx = np.arange(D)[:, None]
    d_idx = np.arange(D)[None, :]
    j = c_idx - d_idx + PAD                              # [c, d]
    valid = (j >= 0) & (j < KW)
    jc = np.clip(j, 0, KW - 1)
    for i in range(LC):
        w = conv_w[i, 0, 0]                              # [KW, KW] (s, j)
        for s in range(KW):
            hst[i, D * s:D * (s + 1), :] = np.where(valid, w[s][jc], 0.0)
        hst[i, KW * D, :] = conv_b[i]
    shared["HST"] = hst.astype(BF16_NP)

    shared["WA"] = np.concatenate([Wa_w.T, Wa_b[None, :]], 0).astype(BF16_NP)
    shared["WO"] = np.stack(
        [np.concatenate([Wo_w[i].T, Wo_b[i][None, :]], 0) for i in range(LO)])
    shared["WI"] = np.concatenate([Wi_w.T, Wi_b[None, :]], 0)
    shared["SUM8"] = np.tile(np.eye(D, dtype=np.float32), (NCORES, 1))
    shared["ONES"] = np.ones((1, LW), np.float32)
    shared["ONESB"] = np.ones((1, LW), BF16_NP)

    in_maps = []
    for c in range(NCORES):
        blk = slice(NB * c, NB * (c + 1))
        m = dict(shared)
        at = np.ascontiguousarray(A8[blk].T)             # [N, NB] fp8
        m["AT"] = np.ascontiguousarray(
            at.reshape(NT, 128, NB).transpose(1, 0, 2).reshape(128, NT * NB))
        xs_c = np.ones((D + 1, NB), np.float32)
        xs_c[0:D] = xs0[blk].T
        m["XS0"] = xs_c
        in_maps.append(m)
    return in_maps


def run(inputs, trace=False, reps=1):
    from concourse.bass_utils import run_bass_kernel_spmd
    key = ("nc", reps)
    if key not in _CACHE:
        _CACHE[key] = _build_nc(reps)
    in_maps = _prep_inputs(inputs)
    res = run_bass_kernel_spmd(
        _CACHE[key], in_maps, core_ids=list(range(NCORES)), trace=trace)
    out = np.asarray(res.results[0]["OUT"], dtype=np.float32).reshape(1, 2)
    return out, res


def kernel(**inputs) -> np.ndarray:
    out, _ = run(inputs, trace=False)
    return out



# revision 17
# speedup vs baseline: 1.4665x; 1.4665x over previous
"""Trainium2 Bass kernel for nn_CPIPre (GNN message passing + conv/attention).

Strategy (8 NeuronCores, SPMD + collectives):
  - adjacency A [8192, 8192] row-sharded: core c owns rows [1024c, 1024(c+1)).
    Host ships A_blk^T (fp8 e4m3, partition-major [128, 64*1024]) per core; it
    is DMA'd once into SBUF (8MB) and stays resident for all 3 GNN layers
    (memory roofline = read A once at fp8).
  - Per GNN layer: every core computes hs = relu(xs@Wg^T+b) for its own rows
    (tensor engine, bias folded in via an ones-row) directly into fp8 with a
    per-layer power-of-2 scale (hs2 reaches ~1e5; fp8e4m3 max is 448),
    AllGathers hs (fp8, 10KB/rank), then computes its row block of A@hs as 32
    fp8 DoubleRow matmuls per 512-column half (two k-tiles per matmult at 0.5
    cycles/row - 4x the fp8 streaming rate of the plain formulation). The
    psum is folded back as xs_next = psum*2^s + xs_cur in one DVE op.
  - The [1,2] result needs mean(xs3) only, so layer 3 skips the xs update:
    row-partials come from ACT-engine accum_out reads of the psum, and a tiny
    AllGather ([10,1] per core) combines cores.
  - Conv branch (11x11 conv on the [4096,10] word embedding "image") is
    reformulated as one K=111 matmul per 512-column tile against a stacked
    shifted-copies tensor S [111, 4096] built by 11 shift-DMAs; bias via an
    ones-row in S.
  - Attention uses a transposed formulation: hs_a is built both as [10, L]
    (hsa) and chunk-transposed [L, 10] (hsT, 32 PE matmuls against ws3).
    Per 128-word chunk: w^T = tanh(hsa_chunk^T @ h) on PE+ACT ([128,1] ops),
    and protein accumulates as hsT_chunk^T @ w^T into one psum - no DVE
    broadcast/multiply/reduce chain.
  - Everything except the A row-block and index gathers is replicated; the
    final output is read from core 0.
"""
import numpy as np
import ml_dtypes

N = 8192
D = 10
DP = 16                   # hs column dim padded for dual-fp8 ldweights
NCORES = 8
NB = N // NCORES          # 1024 rows per core
NT = N // 128             # 64 contraction tiles
L = 4096
KW = 11
PAD = 5
LW = L + 2 * PAD          # 4106 padded width
LG = LC = LO = 3

# per-layer power-of-2 scaling of hs into fp8e4m3 (max normal 448):
# hs0 max ~0.24 (scale up for subnormal headroom), hs1 max ~113, hs2 max ~1.1e5
HS_SCALE = [2.0 ** 6, 1.0, 2.0 ** -10]
HS_UNSCALE = [2.0 ** -6, 1.0, 2.0 ** 10]

BF16_NP = ml_dtypes.bfloat16
FP8_NP = ml_dtypes.float8_e4m3

_CACHE = {}


def _build_nc(reps=1, single_core=False):
    import concourse.bacc as bacc
    import concourse.mybir as mybir
    from concourse import tile

    F32 = mybir.dt.float32
    BF16 = mybir.dt.bfloat16
    FP8 = mybir.dt.float8e4
    AF = mybir.ActivationFunctionType
    ALU = mybir.AluOpType
    AX = mybir.AxisListType
    PM = mybir.MatmulPerfMode

    ndev = 1 if single_core else NCORES
    nc = bacc.Bacc("TRN2", target_bir_lowering=False, debug=False,
                   num_devices=ndev)

    at_d = nc.dram_tensor("AT", [128, NT * NB], FP8, kind="ExternalInput")
    xs0_d = nc.dram_tensor("XS0", [D + 1, NB], F32, kind="ExternalInput")
    s0_d = nc.dram_tensor("S0", [KW * D + 1, L], BF16, kind="ExternalInput")
    wg_d = nc.dram_tensor("WG", [LG, D + 1, D], F32, kind="ExternalInput")
    hst_d = nc.dram_tensor("HST", [LC, KW * D + 1, D], BF16, kind="ExternalInput")
    wa_d = nc.dram_tensor("WA", [D + 1, D], BF16, kind="ExternalInput")
    wo_d = nc.dram_tensor("WO", [LO, 2 * D + 1, 2 * D], F32, kind="ExternalInput")
    wi_d = nc.dram_tensor("WI", [2 * D + 1, 2], F32, kind="ExternalInput")
    sum8_d = nc.dram_tensor("SUM8", [NCORES * D, D], F32, kind="ExternalInput")
    ones_d = nc.dram_tensor("ONES", [1, LW], F32, kind="ExternalInput")
    onesb_d = nc.dram_tensor("ONESB", [1, LW], BF16, kind="ExternalInput")
    out_d = nc.dram_tensor("OUT", [2, 1], F32, kind="ExternalOutput")

    rg = [list(range(NCORES))]

    with tile.TileContext(nc) as tc:
        with (
            tc.tile_pool(name="const", bufs=1) as cp,
            tc.tile_pool(name="work", bufs=2) as wp,
            tc.tile_pool(name="pbig", bufs=2, space="PSUM") as pp_big,
            tc.tile_pool(name="psmall", bufs=1, space="PSUM") as pp_small,
            tc.tile_pool(name="pconv", bufs=2, space="PSUM") as pp_conv,
            tc.tile_pool(name="pattn", bufs=2, space="PSUM") as pp_attn,
            tc.tile_pool(name="pacc", bufs=1, space="PSUM") as pp_acc,
            tc.tile_pool(name="dram", bufs=1, space="DRAM") as dp,
        ):
          for _rep in range(reps):
            at_sb = cp.tile([128, NT * NB], FP8)

            # ---------------- constants (emitted BEFORE the A load so their
            # DMAs drain first and PE can start smalls/conv/AG0 immediately) --
            wg_sb = cp.tile([D + 1, LG * D], F32)
            nc.sync.dma_start(wg_sb.rearrange("k (i d) -> k i d", d=D),
                              wg_d.rearrange("i k d -> k i d"))
            hst_sb = cp.tile([KW * D + 1, LC * D], BF16)
            nc.sync.dma_start(hst_sb.rearrange("k (i d) -> k i d", d=D),
                              hst_d.rearrange("i k d -> k i d"))
            wa_sb = cp.tile([D + 1, D], BF16)
            nc.sync.dma_start(wa_sb[:, :], wa_d[:, :])
            wo_sb = cp.tile([2 * D + 1, LO * 2 * D], F32)
            nc.sync.dma_start(wo_sb.rearrange("k (i d) -> k i d", d=2 * D),
                              wo_d.rearrange("i k d -> k i d"))
            wi_sb = cp.tile([2 * D + 1, 2], F32)
            nc.sync.dma_start(wi_sb[:, :], wi_d[:, :])
            sum8_sb = cp.tile([NCORES * D, D], F32)
            nc.sync.dma_start(sum8_sb[:, :], sum8_d[:, :])

            xs_cur = wp.tile([D + 1, NB], F32, name="xs0", tag="xs")
            nc.sync.dma_start(xs_cur[:, :], xs0_d[:, :])

            # conv buffers + S0 load (small, still ahead of the A bulk load)
            s_sb = cp.tile([KW * D + 1, L], BF16)
            ws1_sb = cp.tile([D, LW], BF16)
            ws2_sb = cp.tile([D, LW], BF16)
            ws3_sb = cp.tile([D + 1, LW], BF16)
            for w_sb in (ws1_sb, ws2_sb):
                nc.vector.memset(w_sb[:, 0:PAD], 0.0)
                nc.vector.memset(w_sb[:, PAD + L:LW], 0.0)
            nc.sync.dma_start(s_sb[:, :], s0_d[:, :])
            nc.sync.dma_start(ws3_sb[D:D + 1, PAD:PAD + L], onesb_d[0:1, 0:L])

            # ---------------- A^T resident bulk load (16 chunks, in the kt
            # order the layer-0 big matmul consumes them) --------------------
            for g in range(16):
                sl = slice(4 * NB * g, 4 * NB * (g + 1))
                nc.sync.dma_start(at_sb[:, sl], at_d[:, sl])
            at3 = at_sb.rearrange("p (kt r) -> p kt r", r=NB)

            # attention / MLP buffers
            hsa_sb = cp.tile([D, L], BF16)
            hsT_sb = cp.tile([128, 32 * D], BF16)
            wtT_sb = cp.tile([128, 32], BF16)
            part9 = cp.tile([D, 3], F32)
            ppart = cp.tile([D, 1], F32)
            accscr = cp.tile([D, 1024], BF16)
            hv = cp.tile([D, 1], BF16)
            cvec = cp.tile([D + 1, 1], BF16)
            catv = cp.tile([2 * D + 1, LO + 1], F32)
            res_sb = cp.tile([2, 1], F32)
            nc.sync.dma_start(cvec[D:D + 1, :], onesb_d[0:1, 0:1])
            nc.sync.dma_start(catv[2 * D:2 * D + 1, :], ones_d[0:1, 0:LO + 1])

            ws_srcs = [None, ws1_sb, ws2_sb, ws3_sb]

            def conv_layer(i):
                src = ws_srcs[i]
                dst = ws_srcs[i + 1]
                if i > 0:
                    for s in range(KW):
                        nc.sync.dma_start(s_sb[D * s:D * (s + 1), 0:L],
                                          src[0:D, s:s + L])
                for half in range(2):
                    pc = pp_conv.tile([128, 512], F32,
                                      name=f"ps_c{i}_{half}", tag="conv")
                    for q in range(4):
                        nt = 4 * half + q
                        nc.tensor.matmul(
                            pc[32 * q:32 * q + D, :],
                            hst_sb[:, D * i:D * (i + 1)],
                            s_sb[:, 512 * nt:512 * (nt + 1)],
                            start=True, stop=True,
                            tile_position=(0, 32 * q),
                        )
                    for q in range(4):
                        nt = 4 * half + q
                        nc.scalar.activation(
                            dst[0:D, PAD + 512 * nt:PAD + 512 * (nt + 1)],
                            pc[32 * q:32 * q + D, :], AF.Relu)

            # ---------------- GNN layers ------------------------------------
            for i in range(LG):
                # hs for my rows: 8 matmuls [128,10] + relu -> scaled fp8
                ps_s = pp_small.tile([128, 8 * D], F32,
                                     name=f"ps_s{i}", tag="small")
                for t in range(8):
                    nc.tensor.matmul(
                        ps_s[:, D * t:D * (t + 1)],
                        xs_cur[:, 128 * t:128 * (t + 1)],
                        wg_sb[:, D * i:D * (i + 1)],
                        start=True, stop=True)
                # hsl in a 16-col-padded layout: dual-fp8 ldweights needs
                # 4-byte-aligned pair strides (cols 10-15 are junk; they land
                # in psum partitions 10-15 which are never read)
                resid = i >= 1
                hsl = wp.tile([128, 8 * DP], FP8, name=f"hsl{i}", tag="hsl")
                nc.vector.memset(
                    hsl.rearrange("p (s d) -> p s d", d=DP)[:, :, D:DP], 0.0)
                nc.scalar.activation(
                    hsl.rearrange("p (s d) -> p s d", d=DP)[:, :, 0:D],
                    ps_s.rearrange("p (s d) -> p s d", d=D),
                    AF.Relu, scale=HS_SCALE[i])
                if resid:
                    # fp8 residual h2 = fp8(H - fp8(H)) at the SAME scale:
                    # fp8's floating exponent makes the ~2^-4-smaller residual
                    # precise without a second unscale pass, so A@h1 + A@h2
                    # accumulates in one psum group (layers 1-2 lose too much
                    # to a single fp8 hs; layer 0 does not)
                    tmp_bf = wp.tile([128, 8 * D], BF16,
                                     name=f"tmpb{i}", tag="tmpb")
                    nc.scalar.activation(tmp_bf[:, :], ps_s[:, :], AF.Relu,
                                         scale=HS_SCALE[i])
                    diff = wp.tile([128, 8 * D], BF16,
                                   name=f"diff{i}", tag="diff")
                    nc.vector.tensor_sub(
                        diff[:, :], tmp_bf[:, :],
                        hsl.rearrange("p (s d) -> p s d", d=DP)[:, :, 0:D])
                    hsl2 = wp.tile([128, 8 * DP], FP8,
                                   name=f"hsl2_{i}", tag="hsl2")
                    nc.vector.memset(
                        hsl2.rearrange("p (s d) -> p s d", d=DP)[:, :, D:DP],
                        0.0)
                    nc.scalar.activation(
                        hsl2.rearrange("p (s d) -> p s d", d=DP)[:, :, 0:D],
                        diff.rearrange("p (s d) -> p s d", d=D), AF.Copy)

                if i == LG - 1:
                    # xs2 row-partials (DVE) early, off the critical tail
                    nc.vector.reduce_sum(part9[:, 0:1], xs_cur[0:D, :],
                                         axis=AX.X)

                cw = 8 * DP * (2 if resid else 1)
                cc_in = dp.tile([128, cw], FP8,
                                name=f"cci{i}", tag=f"cci{i}")
                nc.sync.dma_start(cc_in[:, 0:8 * DP], hsl[:, :])
                if resid:
                    nc.sync.dma_start(cc_in[:, 8 * DP:cw], hsl2[:, :])
                cc_out = dp.tile([128 * NCORES, cw], FP8,
                                 name=f"cco{i}", tag=f"cco{i}",
                                 addr_space="Local" if single_core else "Shared")
                if single_core:
                    for r in range(NCORES):
                        nc.sync.dma_start(cc_out[128 * r:128 * (r + 1), :],
                                          cc_in[:, :])
                else:
                    nc.gpsimd.collective_compute(
                        "AllGather", ALU.bypass, replica_groups=rg,
                        ins=[cc_in.opt()], outs=[cc_out.opt()])
                hs_sb = wp.tile([128, NT * DP], FP8, name=f"hs{i}", tag="hs")
                nc.sync.dma_start(
                    hs_sb.rearrange("p (r f) -> p r f", f=8 * DP),
                    cc_out[:, 0:8 * DP].rearrange("(r p) f -> p r f", p=128))
                hs3 = hs_sb.rearrange("p (kt d) -> p kt d", d=DP)
                if resid:
                    hs2_sb = wp.tile([128, NT * DP], FP8,
                                     name=f"hs2_{i}", tag="hs2")
                    nc.sync.dma_start(
                        hs2_sb.rearrange("p (r f) -> p r f", f=8 * DP),
                        cc_out[:, 8 * DP:cw].rearrange("(r p) f -> p r f",
                                                       p=128))
                    hs23 = hs2_sb.rearrange("p (kt d) -> p kt d", d=DP)

                # conv layer emitted here to interleave PE work
                if i < LC:
                    conv_layer(i)

                # big matmul: out^T [10, 1024] += hs_kt^T @ A^T_kt as fp8
                # DoubleRow pairs (2 k-tiles per matmult, 0.5 cycles/row).
                # Pair-major order so layer 0 streams behind the A chunk DMAs.
                pb = [pp_big.tile([128, 512], F32, name=f"ps_b{i}_{h}",
                                  tag="big") for h in range(2)]
                for j in range(NT // 2):
                    for h in range(2):
                        nc.tensor.matmul(
                            pb[h][0:DP, :],
                            hs3[:, 2 * j:2 * j + 2, :],
                            at3[:, 2 * j:2 * j + 2, 512 * h:512 * (h + 1)],
                            start=(j == 0),
                            stop=(not resid and j == NT // 2 - 1),
                            perf_mode=PM.DoubleRow,
                        )
                if resid:
                    for j in range(NT // 2):
                        for h in range(2):
                            nc.tensor.matmul(
                                pb[h][0:DP, :],
                                hs23[:, 2 * j:2 * j + 2, :],
                                at3[:, 2 * j:2 * j + 2,
                                    512 * h:512 * (h + 1)],
                                start=False, stop=(j == NT // 2 - 1),
                                perf_mode=PM.DoubleRow,
                            )

                if i < LG - 1:
                    xs_next = wp.tile([D + 1, NB], F32,
                                      name=f"xs{i + 1}", tag="xs")
                    nc.sync.dma_start(xs_next[D:D + 1, :], ones_d[0:1, 0:NB])
                    for h in range(2):
                        sl = slice(512 * h, 512 * (h + 1))
                        nc.vector.scalar_tensor_tensor(
                            xs_next[0:D, sl], pb[h][0:D, :], HS_UNSCALE[i],
                            xs_cur[0:D, sl], ALU.mult, ALU.add)
                    xs_cur = xs_next
                else:
                    # psum row-partials on ACT (accum_out = free-axis sum)
                    for h in range(2):
                        nc.scalar.activation(
                            accscr[:, 512 * h:512 * (h + 1)], pb[h][0:D, :],
                            AF.Copy, scale=HS_UNSCALE[i],
                            accum_out=part9[:, 1 + h:2 + h])
                    nc.vector.reduce_sum(ppart[:, :], part9[:, :], axis=AX.X)

            # hsa [10, L] and chunk-transposed hsT [L, 10] from ws3 (conv L2);
            # own psum tags so they are not serialized behind the cc2
            # collective by psum slot reuse
            for nt in range(8):
                pa = pp_conv.tile([128, 512], F32, name=f"ps_a{nt}", tag="conv")
                nc.tensor.matmul(pa[0:D, :], wa_sb[:, :],
                                 ws3_sb[:, PAD + 512 * nt:PAD + 512 * (nt + 1)],
                                 start=True, stop=True)
                nc.scalar.activation(hsa_sb[:, 512 * nt:512 * (nt + 1)],
                                     pa[0:D, :], AF.Relu)
            for c in range(32):
                pt = pp_attn.tile([128, D], F32, name=f"ps_t{c}", tag="attn")
                nc.tensor.matmul(pt[:, :],
                                 ws3_sb[:, PAD + 128 * c:PAD + 128 * (c + 1)],
                                 wa_sb[:, :], start=True, stop=True)
                nc.scalar.activation(hsT_sb[:, D * c:D * (c + 1)], pt[:, :],
                                     AF.Relu)

            # ---------------- compound via tiny AllGather -------------------
            cc2_in = dp.tile([D, 1], F32, name="cc2i", tag="cc2i")
            nc.sync.dma_start(cc2_in[:, :], ppart[:, :])
            cc2_out = dp.tile([NCORES * D, 1], F32, name="cc2o", tag="cc2o",
                              addr_space="Local" if single_core else "Shared")
            if single_core:
                for r in range(NCORES):
                    nc.sync.dma_start(cc2_out[D * r:D * (r + 1), :],
                                      cc2_in[:, :])
            else:
                nc.gpsimd.collective_compute(
                    "AllGather", ALU.bypass, replica_groups=rg,
                    ins=[cc2_in.opt()], outs=[cc2_out.opt()])
            pc_sb = cp.tile([NCORES * D, 1], F32)
            nc.sync.dma_start(pc_sb[:, :], cc2_out[:, :])
            ps_cmp = pp_attn.tile([D, 1], F32, name="ps_cmp", tag="attn")
            nc.tensor.matmul(ps_cmp[:, :], sum8_sb[:, :], pc_sb[:, :],
                             start=True, stop=True)
            nc.scalar.activation(cvec[0:D, :], ps_cmp[:, :], AF.Copy,
                                 scale=1.0 / N)
            nc.scalar.activation(catv[0:D, 0:1], ps_cmp[:, :], AF.Copy,
                                 scale=1.0 / N)

            # ---------------- attention (post-AG part) ----------------------
            ph = pp_attn.tile([D, 1], F32, name="ps_h", tag="attn")
            nc.tensor.matmul(ph[:, :], wa_sb[:, :], cvec[:, :],
                             start=True, stop=True)
            nc.scalar.activation(hv[:, :], ph[:, :], AF.Relu)

            # per 128-word chunk: wtT = tanh(hsa_chunk^T @ h), then
            # protein_psum += hsT_chunk^T @ wtT  (all [<=128,1] ops)
            ppro_ps = pp_acc.tile([D, 1], F32, name="ps_pro", tag="acc")
            for c in range(32):
                pw = pp_attn.tile([128, 1], F32, name=f"ps_w{c}", tag="attn")
                nc.tensor.matmul(pw[:, :], hsa_sb[:, 128 * c:128 * (c + 1)],
                                 hv[:, :], start=True, stop=True)
                nc.scalar.activation(wtT_sb[:, c:c + 1], pw[:, :], AF.Tanh)
                nc.tensor.matmul(ppro_ps[:, :], hsT_sb[:, D * c:D * (c + 1)],
                                 wtT_sb[:, c:c + 1],
                                 start=(c == 0), stop=(c == 31))
            ppro2 = cp.tile([D, 1], F32)
            nc.scalar.activation(ppro2[:, :], ppro_ps[:, :], AF.Copy,
                                 scale=1.0 / L)
            nc.sync.dma_start(catv[D:2 * D, 0:1], ppro2[:, :])

            # ---------------- output MLP ------------------------------------
            for j in range(LO):
                pm = pp_attn.tile([2 * D, 1], F32, name=f"ps_m{j}", tag="attn")
                nc.tensor.matmul(pm[:, :], wo_sb[:, 2 * D * j:2 * D * (j + 1)],
                                 catv[:, j:j + 1], start=True, stop=True)
                nc.scalar.activation(catv[0:2 * D, j + 1:j + 2], pm[:, :],
                                     AF.Relu)
            pf = pp_attn.tile([2, 1], F32, name="ps_f", tag="attn")
            nc.tensor.matmul(pf[:, :], wi_sb[:, :], catv[:, LO:LO + 1],
                             start=True, stop=True)
            nc.scalar.activation(res_sb[:, :], pf[:, :], AF.Copy)
            nc.sync.dma_start(out_d[:, :], res_sb[:, :])

    nc.compile()
    return nc


def _prep_inputs(inputs):
    fp = np.asarray(inputs["fingerprints"]).astype(np.int64)
    A = np.ascontiguousarray(np.asarray(inputs["adjacency"], dtype=np.float32))
    words = np.asarray(inputs["words"]).astype(np.int64)
    emb_fp = np.asarray(inputs["emb_fp"], dtype=np.float32)
    emb_word = np.asarray(inputs["emb_word"], dtype=np.float32)
    Wg_w = np.asarray(inputs["Wg_w"], dtype=np.float32)
    Wg_b = np.asarray(inputs["Wg_b"], dtype=np.float32)
    conv_w = np.asarray(inputs["conv_w"], dtype=np.float32)
    conv_b = np.asarray(inputs["conv_b"], dtype=np.float32)
    Wa_w = np.asarray(inputs["Wa_w"], dtype=np.float32)
    Wa_b = np.asarray(inputs["Wa_b"], dtype=np.float32)
    Wo_w = np.asarray(inputs["Wo_w"], dtype=np.float32)
    Wo_b = np.asarray(inputs["Wo_b"], dtype=np.float32)
    Wi_w = np.asarray(inputs["Wi_w"], dtype=np.float32)
    Wi_b = np.asarray(inputs["Wi_b"], dtype=np.float32)

    xs0 = emb_fp[fp]                                     # [N, D]
    ws0 = emb_word[words]                                # [L, D]

    A8 = A.astype(FP8_NP)
    shared = {}
    ws0_pad = np.zeros((D, LW), np.float32)
    ws0_pad[:, PAD:PAD + L] = ws0.T
    s0 = np.ones((KW * D + 1, L), np.float32)
    for s in range(KW):
        s0[D * s:D * (s + 1)] = ws0_pad[:, s:s + L]
    shared["S0"] = s0.astype(BF16_NP)

    shared["WG"] = np.stack(
        [np.concatenate([Wg_w[i].T, Wg_b[i][None, :]], 0) for i in range(LG)])

    hst = np.zeros((LC, KW * D + 1, D), np.float32)  # cast to bf16 below
    c_idx = np.arange(D)[:, None]
    d_idx = np.arange(D)[None, :]
    j = c_idx - d_idx + PAD                              # [c, d]
    valid = (j >= 0) & (j < KW)
    jc = np.clip(j, 0, KW - 1)
    for i in range(LC):
        w = conv_w[i, 0, 0]                              # [KW, KW] (s, j)
        for s in range(KW):
            hst[i, D * s:D * (s + 1), :] = np.where(valid, w[s][jc], 0.0)
        hst[i, KW * D, :] = conv_b[i]
    shared["HST"] = hst.astype(BF16_NP)

    shared["WA"] = np.concatenate([Wa_w.T, Wa_b[None, :]], 0).astype(BF16_NP)
    shared["WO"] = np.stack(
        [np.concatenate([Wo_w[i].T, Wo_b[i][None, :]], 0) for i in range(LO)])
    shared["WI"] = np.concatenate([Wi_w.T, Wi_b[None, :]], 0)
    shared["SUM8"] = np.tile(np.eye(D, dtype=np.float32), (NCORES, 1))
    shared["ONES"] = np.ones((1, LW), np.float32)
    shared["ONESB"] = np.ones((1, LW), BF16_NP)

    in_maps = []
    for c in range(NCORES):
        blk = slice(NB * c, NB * (c + 1))
        m = dict(shared)
        at = np.ascontiguousarray(A8[blk].T)             # [N, NB] fp8
        m["AT"] = np.ascontiguousarray(
            at.reshape(NT, 128, NB).transpose(1, 0, 2).reshape(128, NT * NB))
        xs_c = np.ones((D + 1, NB), np.float32)
        xs_c[0:D] = xs0[blk].T
        m["XS0"] = xs_c
        in_maps.append(m)
    return in_maps


def run(inputs, trace=False, reps=1):
    from concourse.bass_utils import run_bass_kernel_spmd
    key = ("nc", reps)
    if key not in _CACHE:
        _CACHE[key] = _build_nc(reps)
    in_maps = _prep_inputs(inputs)
    res = run_bass_kernel_spmd(
        _CACHE[key], in_maps, core_ids=list(range(NCORES)), trace=trace)
    out = np.asarray(res.results[0]["OUT"], dtype=np.float32).reshape(1, 2)
    return out, res


def kernel(**inputs) -> np.ndarray:
    out, _ = run(inputs, trace=False)
    return out


# revision 58
# speedup vs baseline: 2.8413x; 1.9375x over previous
"""Trainium2 Bass kernel for nn_CPIPre (GNN message passing + conv/attention).

Strategy (8 NeuronCores, SPMD + collectives):
  - adjacency A [8192, 8192] row-sharded: core c owns rows [1024c, 1024(c+1)).
    Host ships A_blk^T (fp8 e4m3, partition-major [128, 64*1024]) per core; it
    is DMA'd once into SBUF (8MB) and stays resident for all 3 GNN layers
    (memory roofline = read A once at fp8).
  - Per GNN layer: every core computes hs = relu(xs@Wg^T+b) for its own rows
    (tensor engine, bias folded in via an ones-row) directly into fp8 with a
    per-layer power-of-2 scale (hs2 reaches ~1e5; fp8e4m3 max is 448), plus an
    fp8 residual h2 = fp8(H - fp8(H)) on layers 1-2 (a single fp8 hs loses too
    much there; the residual's floating exponent makes it precise at the same
    scale, so A@h1 + A@h2 accumulate in ONE psum group). hs columns are padded
    to 16 (dual-fp8 ldweights needs 4-byte-aligned pair strides; pad columns
    land in psum partitions 10-15 which are never read).
  - h1||h2 AllGather (fp8, one 16/32KB payload), then the row block of A@hs
    runs as fp8 DoubleRow matmuls: two k-tiles per matmult at 0.5 cycles/row,
    4x the streaming rate of the plain fp8 formulation. xs_next folds back as
    psum*2^s + xs_cur in one DVE op per half.
  - The [1,2] result needs mean(xs3) only, so layer 3 skips the xs update:
    row-partials via ACT-engine accum_out reads of the psum, a tiny AllGather
    ([10,1] per core) combines cores.
  - Conv branch (11x11 conv on the [4096,10] word embedding "image") is one
    K=111 matmul per 512-column tile against stacked shifted copies S
    [111, 4096], rows ordered (channel, tap) so each layer's S rebuild is ONE
    overlapping-window DMA (src AP dim [stride 1, count 11]); bias via an
    ones-row in S.
  - Attention uses a transposed formulation: hs_a built as [10, L] (hsa) and
    chunk-transposed [L, 10] (hsT). All 32 w^T-chunk matmuls accumulate into
    one zeroed psum tile (skip_group_check), ONE tanh act covers all of w^T,
    then 32 back-to-back matmuls accumulate protein = hsT^T @ w^T - no
    per-chunk PE<->ACT ping-pong on the critical tail.
  - Collectives in the single-core timing surrogate are one broadcast-AP DMA
    each; the SPMD path uses real AllGathers.
  - Emission is ordered so every in-order engine queue matches data readiness:
    A loads as 16 chunk tiles (streamed by the L0 bigs), the conv branch's
    compute is pinned into the AllGather gaps via tile_wait_until (PE idles
    there and stays warm for the DVFS ramp), and the S0 upload runs after the
    A bulk (conv L0 is not needed until ~30us).
  - Everything except the A row-block and index gathers is replicated; the
    final output is read from core 0.
"""
import numpy as np
import ml_dtypes

N = 8192
D = 10
DP = 16                   # hs column dim padded for dual-fp8 ldweights
NCORES = 8
NB = N // NCORES          # 1024 rows per core
NT = N // 128             # 64 contraction tiles
L = 4096
KW = 11
PAD = 5
LW = L + 2 * PAD          # 4106 padded width
LG = LC = LO = 3

# per-layer power-of-2 scaling of hs into fp8e4m3 (max normal 448):
# hs0 max ~0.24 (scale up for subnormal headroom), hs1 max ~113, hs2 max ~1.1e5
HS_SCALE = [2.0 ** 6, 1.0, 2.0 ** -10]
HS_UNSCALE = [2.0 ** -6, 1.0, 2.0 ** 10]

# packed f32 const blob CONSTF [80, 102]: WG 0:30, WO 30:90, WI 90:92,
# SUM8 92:102. packed bf16 blob CONSTB [111, 40]: HST 0:30, WA 30:40.
CF_WG, CF_WO, CF_WI, CF_SUM8, CF_WOC, CF_WOP, CF_W = 0, 30, 90, 92, 102, 122, 142
CB_HST, CB_WA, CB_W = 0, 30, 40

BF16_NP = ml_dtypes.bfloat16
FP8_NP = ml_dtypes.float8_e4m3

_CACHE = {}

# emission-order tuning knobs (see sweep.py)
CONV_PLACEMENT = "late"     # "interleave" | "late"
DVE_RELU_SPLIT = False
HST_BATCH = False
WAIT_CONV0 = 0.0285
WAIT_CONV1 = 0.034
WAIT_CONV2 = 0.058
WAIT_ATTN = 0.060
CHUNKS_FIRST = 4


def _build_nc(reps=1, single_core=False):
    import concourse.bacc as bacc
    import concourse.mybir as mybir
    from concourse import tile

    F32 = mybir.dt.float32
    BF16 = mybir.dt.bfloat16
    FP8 = mybir.dt.float8e4
    AF = mybir.ActivationFunctionType
    ALU = mybir.AluOpType
    AX = mybir.AxisListType
    PM = mybir.MatmulPerfMode

    ndev = 1 if single_core else NCORES
    nc = bacc.Bacc("TRN2", target_bir_lowering=False, debug=False,
                   num_devices=ndev)

    at_d = nc.dram_tensor("AT", [128, NT * NB], FP8, kind="ExternalInput")
    xs0_d = nc.dram_tensor("XS0", [D + 1, NB], F32, kind="ExternalInput")
    s0_d = nc.dram_tensor("S0", [KW * D + 1, L], BF16, kind="ExternalInput")
    cf_d = nc.dram_tensor("CONSTF", [NCORES * D, CF_W], F32,
                          kind="ExternalInput")
    cb_d = nc.dram_tensor("CONSTB", [KW * D + 1, CB_W], BF16,
                          kind="ExternalInput")
    onesb_d = nc.dram_tensor("ONESB", [1, L], BF16, kind="ExternalInput")
    onesf_d = nc.dram_tensor("ONESF", [1, NB], F32, kind="ExternalInput")
    out_d = nc.dram_tensor("OUT", [2, 1], F32, kind="ExternalOutput")

    rg = [list(range(NCORES))]

    def window11(ap):
        """Insert an overlapping dim [stride 1 elem, count KW] after the
        partition dim: [p, x] view -> iteration (p, s, x) reading p, s+x."""
        w = ap.unsqueeze(1)
        w.ap[1] = [1, KW]
        return w

    with tile.TileContext(nc) as tc:
        with (
            tc.tile_pool(name="const", bufs=1) as cp,
            tc.tile_pool(name="work", bufs=2) as wp,
            tc.tile_pool(name="pbig", bufs=2, space="PSUM") as pp_big,
            tc.tile_pool(name="psmall", bufs=1, space="PSUM") as pp_small,
            tc.tile_pool(name="pconv", bufs=2, space="PSUM") as pp_conv,
            tc.tile_pool(name="pattn", bufs=2, space="PSUM") as pp_attn,
            tc.tile_pool(name="pacc", bufs=1, space="PSUM") as pp_acc,
            tc.tile_pool(name="dram", bufs=1, space="DRAM") as dp,
        ):
          for _rep in range(reps):
            # ---------------- constants (emitted BEFORE the A load so their
            # DMAs drain first and PE can start smalls/conv/AG0 immediately) --
            xs_cur0 = wp.tile([D + 1, NB], F32, name="xs0", tag="xs")
            nc.sync.dma_start(xs_cur0[:, :], xs0_d[:, :])
            cf_sb = cp.tile([NCORES * D, CF_W], F32)
            nc.sync.dma_start(cf_sb[:, :], cf_d[:, :])
            at_t = []

            def load_chunks(g0, g1):
                for g in range(g0, g1):
                    sl = slice(4 * NB * g, 4 * NB * (g + 1))
                    t = cp.tile([128, 4 * NB], FP8, name=f"at{g}",
                                tag=f"at{g}")
                    nc.sync.dma_start(t[:, :], at_d[:, sl])
                    at_t.append(t.rearrange("p (kt r) -> p kt r", r=NB))

            load_chunks(0, CHUNKS_FIRST)
            s_sb = cp.tile([KW * D + 1, L], BF16)
            cb_sb = cp.tile([KW * D + 1, CB_W], BF16)
            nc.sync.dma_start(cb_sb[:, :], cb_d[:, :])

            def wg_sl(i):
                return cf_sb[0:D + 1, CF_WG + D * i:CF_WG + D * (i + 1)]

            def wo_sl(j):
                return cf_sb[0:2 * D + 1, CF_WO + 2 * D * j:CF_WO + 2 * D * (j + 1)]

            wi_sl = cf_sb[0:2 * D + 1, CF_WI:CF_WI + 2]
            sum8_sl = cf_sb[:, CF_SUM8:CF_SUM8 + D]

            def hst_sl(i):
                return cb_sb[:, CB_HST + D * i:CB_HST + D * (i + 1)]

            wa_sl = cb_sb[0:D + 1, CB_WA:CB_WA + D]

            xs_cur = xs_cur0

            # conv buffers
            ws1_sb = cp.tile([D, LW], BF16)
            ws2_sb = cp.tile([D, LW], BF16)
            ws3_sb = cp.tile([D + 1, LW], BF16)
            for w_sb in (ws1_sb, ws2_sb):
                nc.vector.memset(w_sb[:, 0:PAD], 0.0)
                nc.vector.memset(w_sb[:, PAD + L:LW], 0.0)
            nc.sync.dma_start(ws3_sb[D:D + 1, PAD:PAD + L], onesb_d[0:1, 0:L])

            # ---------------- A^T resident bulk load (8 chunks as SEPARATE
            # tiles so the layer-0 big matmul streams chunk-by-chunk instead
            # of waiting on one whole-tile dependency) ------------------------
            load_chunks(CHUNKS_FIRST, 16)
            # S0 after the A bulk: conv L0's compute isn't needed until ~30us
            # (it only gates conv L1 via ws1 -> window1)
            nc.sync.dma_start(s_sb[:, :], s0_d[:, :])

            def at_sl(j, h):
                # moving operand for k-tile pair j, column half h
                return at_t[j // 2][:, 2 * (j % 2):2 * (j % 2) + 2,
                                    512 * h:512 * (h + 1)]

            # attention / MLP buffers
            hsa_sb = cp.tile([D, L], BF16)
            hsT_sb = cp.tile([128, 32 * D], BF16)
            wtT_sb = cp.tile([128, 32], BF16)
            part9 = cp.tile([D, 3], F32)
            ppart = cp.tile([D, 1], F32)
            accscr = cp.tile([D, 1024], BF16)
            hv = cp.tile([D, 1], BF16)
            cvec = cp.tile([D + 1, 1], BF16)
            catv = cp.tile([2 * D + 1, LO + 1], F32)
            res_sb = cp.tile([2, 1], F32)
            ppro2 = cp.tile([D, 1], F32)
            pc_sb = cp.tile([D, NCORES], F32)
            cv32 = cp.tile([D + 1, 1], F32)
            # ones rows live at partitions 10/20 where compute engines may
            # not start a write - load them via DMA
            nc.scalar.dma_start(cvec[D:D + 1, :], onesb_d[0:1, 0:1])
            nc.scalar.dma_start(cv32[D:D + 1, :], onesf_d[0:1, 0:1])
            nc.scalar.dma_start(catv[2 * D:2 * D + 1, :],
                                onesf_d[0:1, 0:LO + 1])

            ws_srcs = [None, ws1_sb, ws2_sb, ws3_sb]

            def conv_window(i):
                # rebuild S with ONE overlapping-window DMA: S[(c s), x] =
                # src[c, s + x] (S rows are (channel, tap)-ordered)
                nc.scalar.dma_start(s_sb[0:KW * D, 0:L],
                                    window11(ws_srcs[i][0:D, 0:L]))

            def conv_layer(i):
                dst = ws_srcs[i + 1]
                for half in range(2):
                    pc = pp_conv.tile([128, 512], F32,
                                      name=f"ps_c{i}_{half}", tag="conv")
                    for q in range(4):
                        nt = 4 * half + q
                        nc.tensor.matmul(
                            pc[32 * q:32 * q + D, :],
                            hst_sl(i),
                            s_sb[:, 512 * nt:512 * (nt + 1)],
                            start=True, stop=True,
                            tile_position=(0, 32 * q),
                        )
                    # relus optionally split across ACT and the idle DVE
                    for q in range(4):
                        nt = 4 * half + q
                        o = dst[0:D, PAD + 512 * nt:PAD + 512 * (nt + 1)]
                        if not DVE_RELU_SPLIT or q % 2 == 0:
                            nc.scalar.activation(
                                o, pc[32 * q:32 * q + D, :], AF.Relu)
                        else:
                            nc.vector.tensor_scalar_max(
                                o, pc[32 * q:32 * q + D, :], 0.0)

            # ---------------- GNN layers, hand-interleaved with the conv
            # branch so the PE queue order matches data readiness ------------
            xs_tiles = [xs_cur]
            hs_alls = {}
            pbs = {}

            def emit_smalls_ag(i):
                resid = i >= 1
                cw = 8 * DP * (2 if resid else 1)
                xs = xs_tiles[i]
                ps_s = pp_small.tile([128, 8 * D], F32,
                                     name=f"ps_s{i}", tag="small")
                for t in range(8):
                    nc.tensor.matmul(
                        ps_s[:, D * t:D * (t + 1)],
                        xs[:, 128 * t:128 * (t + 1)],
                        wg_sl(i),
                        start=True, stop=True)
                hsl = wp.tile([128, cw], FP8, name=f"hsl{i}", tag="hsl")
                h1v = hsl[:, 0:8 * DP].rearrange("p (s d) -> p s d", d=DP)
                nc.vector.memset(h1v[:, :, D:DP], 0.0)
                nc.scalar.activation(h1v[:, :, 0:D],
                                     ps_s.rearrange("p (s d) -> p s d", d=D),
                                     AF.Relu, scale=HS_SCALE[i])
                if resid:
                    # fp8 residual at the SAME scale (see module docstring)
                    tmp_bf = wp.tile([128, 8 * D], BF16,
                                     name=f"tmpb{i}", tag="tmpb")
                    nc.scalar.activation(tmp_bf[:, :], ps_s[:, :], AF.Relu,
                                         scale=HS_SCALE[i])
                    diff = wp.tile([128, 8 * D], BF16,
                                   name=f"diff{i}", tag="diff")
                    nc.vector.tensor_sub(diff[:, :], tmp_bf[:, :],
                                         h1v[:, :, 0:D])
                    h2v = hsl[:, 8 * DP:cw].rearrange("p (s d) -> p s d",
                                                      d=DP)
                    nc.vector.memset(h2v[:, :, D:DP], 0.0)
                    nc.scalar.activation(
                        h2v[:, :, 0:D],
                        diff.rearrange("p (s d) -> p s d", d=D), AF.Copy)

                if i == LG - 1:
                    # xs2 half-0 row-partials (DVE) early, off the critical
                    # tail (half 1 is folded into the psum stt accum below)
                    nc.vector.reduce_sum(part9[:, 0:1], xs[0:D, 0:512],
                                         axis=AX.X)

                # AllGather h1 and (resid) h2 as separate payloads: the
                # bigs' h1 pass starts while h2 is still in flight
                parts = []
                for g in range(2 if resid else 1):
                    hpart = wp.tile([128, NCORES * 8 * DP], FP8,
                                    name=f"hs{i}_{g}", tag=f"hsg{g}")
                    hsl_g = hsl[:, 8 * DP * g:8 * DP * (g + 1)]
                    if single_core:
                        nc.scalar.dma_start(
                            hpart.rearrange("p (r f) -> p r f", f=8 * DP),
                            hsl_g.unsqueeze(1).broadcast_to(
                                (128, NCORES, 8 * DP)))
                    else:
                        cc_in = dp.tile([128, 8 * DP], FP8,
                                        name=f"cci{i}_{g}", tag=f"cci{i}_{g}")
                        nc.scalar.dma_start(cc_in[:, :], hsl_g)
                        cc_out = dp.tile([128 * NCORES, 8 * DP], FP8,
                                         name=f"cco{i}_{g}",
                                         tag=f"cco{i}_{g}",
                                         addr_space="Shared")
                        nc.gpsimd.collective_compute(
                            "AllGather", ALU.bypass, replica_groups=rg,
                            ins=[cc_in.opt()], outs=[cc_out.opt()])
                        nc.scalar.dma_start(
                            hpart.rearrange("p (r f) -> p r f", f=8 * DP),
                            cc_out.rearrange("(r p) f -> p r f", p=128))
                    parts.append(hpart)
                hs_alls[i] = parts
                if i not in pbs:
                    pbs[i] = [pp_big.tile([128, 512], F32,
                                          name=f"ps_b{i}_{h}", tag="big")
                              for h in range(2)]

            def emit_bigs(i, j0, j1):
                resid = i >= 1
                parts = hs_alls[i]
                pb = pbs[i]

                def pair_sl(j, g):
                    base = (j // 4) * 8 * DP + (j % 4) * 2 * DP
                    return parts[g][:, base:base + 2 * DP].rearrange(
                        "p (two d) -> p two d", d=DP)

                # h-major for layers 1-2 (all chunks resident): half 0's
                # psum stops a full pass early, so its xs-update/accum
                # overlaps half 1's matmuls.  Layer 0 stays j-major to
                # stream behind the chunk DMAs.
                if i == 0:
                    for j in range(j0, j1):
                        for h in range(2):
                            nc.tensor.matmul(
                                pb[h][0:DP, :], pair_sl(j, 0), at_sl(j, h),
                                start=(j == 0),
                                stop=(not resid and j == NT // 2 - 1),
                                perf_mode=PM.DoubleRow,
                            )
                else:
                    for h in range(2):
                        for g in range(2):
                            for j in range(NT // 2):
                                nc.tensor.matmul(
                                    pb[h][0:DP, :], pair_sl(j, g),
                                    at_sl(j, h),
                                    start=(g == 0 and j == 0),
                                    stop=(g == 1 and j == NT // 2 - 1),
                                    perf_mode=PM.DoubleRow,
                                )

            def emit_update(i):
                pb = pbs[i]
                if i < LG - 1:
                    xs_next = wp.tile([D + 1, NB], F32,
                                      name=f"xs{i + 1}", tag="xs")
                    nc.scalar.dma_start(xs_next[D:D + 1, :], onesf_d[0:1, :])
                    for h in range(2):
                        sl = slice(512 * h, 512 * (h + 1))
                        nc.vector.scalar_tensor_tensor(
                            xs_next[0:D, sl], pb[h][0:D, :], HS_UNSCALE[i],
                            xs_tiles[i][0:D, sl], ALU.mult, ALU.add)
                    xs_tiles.append(xs_next)
                else:
                    # psum row-partials: half 0 on ACT (accum_out), half 1 on
                    # DVE (scalar_tensor_tensor with accum_out) in parallel
                    nc.scalar.activation(
                        accscr[:, 0:512], pb[0][0:D, :],
                        AF.Copy, scale=HS_UNSCALE[i],
                        accum_out=part9[:, 1:2])
                    nc.vector.scalar_tensor_tensor(
                        accscr[:, 512:1024], pb[1][0:D, :], HS_UNSCALE[i],
                        xs_tiles[i][0:D, 512:1024], ALU.mult, ALU.add,
                        accum_out=part9[:, 2:3])
                    nc.vector.reduce_sum(ppart[:, :], part9[:, :], axis=AX.X)

            def emit_attn_prep():
                # hsa [10, L] from ws3
                for nt in range(8):
                    pa = pp_conv.tile([128, 512], F32, name=f"ps_a{nt}",
                                      tag="conv")
                    nc.tensor.matmul(
                        pa[0:D, :], wa_sl,
                        ws3_sb[:, PAD + 512 * nt:PAD + 512 * (nt + 1)],
                        start=True, stop=True)
                    if not DVE_RELU_SPLIT or nt % 2 == 0:
                        nc.scalar.activation(
                            hsa_sb[:, 512 * nt:512 * (nt + 1)],
                            pa[0:D, :], AF.Relu)
                    else:
                        nc.vector.tensor_scalar_max(
                            hsa_sb[:, 512 * nt:512 * (nt + 1)],
                            pa[0:D, :], 0.0)
                for c in range(32):
                    pt = pp_attn.tile([128, D], F32, name=f"ps_t{c}",
                                      tag="attn")
                    nc.tensor.matmul(
                        pt[:, :],
                        ws3_sb[:, PAD + 128 * c:PAD + 128 * (c + 1)],
                        wa_sl, start=True, stop=True)
                    nc.scalar.activation(hsT_sb[:, D * c:D * (c + 1)],
                                         pt[:, :], AF.Relu)

            # global order: conv L0 early (fills the chunk-paced phase);
            # conv L1/L2 land in the AllGather gaps after each layer's bigs
            # (PE idles there anyway, and their S-rebuild DMAs are long done);
            # attn prep after L2's bigs fills the cc2-collective gap
            # conv compute is pinned into the AllGather gaps (PE idles
            # there) via wait_until; S-window DMAs issue at natural readiness
            emit_smalls_ag(0)
            with tc.tile_wait_until(WAIT_CONV0):
                conv_layer(0)
            conv_window(1)
            emit_bigs(0, 0, NT // 2)
            emit_update(0)

            emit_smalls_ag(1)
            with tc.tile_wait_until(WAIT_CONV1):
                conv_layer(1)
            emit_bigs(1, 0, NT // 2)
            conv_window(2)
            emit_update(1)

            emit_smalls_ag(2)
            with tc.tile_wait_until(WAIT_CONV2):
                conv_layer(2)
            with tc.tile_wait_until(WAIT_ATTN):
                emit_attn_prep()
            emit_bigs(2, 0, NT // 2)
            emit_update(2)

            # ---------------- compound via tiny AllGather -------------------
            # pc_sb is [d, r] (partition = d) - an SBUF partition dim must not
            # be factored into multiple AP dims, so the rank axis lives in the
            # free dim and a DVE row-sum combines ranks
            if single_core:
                nc.scalar.dma_start(pc_sb[:, :],
                                    ppart[:, 0:1].broadcast_to((D, NCORES)))
            else:
                cc2_in = dp.tile([D, 1], F32, name="cc2i", tag="cc2i")
                nc.scalar.dma_start(cc2_in[:, :], ppart[:, :])
                cc2_out = dp.tile([NCORES * D, 1], F32, name="cc2o",
                                  tag="cc2o", addr_space="Shared")
                nc.gpsimd.collective_compute(
                    "AllGather", ALU.bypass, replica_groups=rg,
                    ins=[cc2_in.opt()], outs=[cc2_out.opt()])
                nc.scalar.dma_start(
                    pc_sb[:, :],
                    cc2_out.rearrange("(r d) x -> d (r x)", d=D))
            ppc = cp.tile([D, 1], F32)
            nc.vector.reduce_sum(ppc[:, :], pc_sb[:, :], axis=AX.X)
            nc.scalar.activation(cvec[0:D, :], ppc[:, :], AF.Copy,
                                 scale=1.0 / N)
            nc.scalar.activation(cv32[0:D, :], ppc[:, :], AF.Copy,
                                 scale=1.0 / N)

            # ---------------- attention (post-AG part) ----------------------
            ph = pp_attn.tile([D, 1], F32, name="ps_h", tag="attn")
            nc.tensor.matmul(ph[:, :], wa_sl, cvec[:, :],
                             start=True, stop=True)
            nc.scalar.activation(hv[:, :], ph[:, :], AF.Relu)

            # all 32 wT-chunk matmuls into one zeroed psum tile, one tanh,
            # then 32 back-to-back matmuls accumulate protein = hsT^T @ wT
            pw32 = pp_attn.tile([128, 32], F32, name="ps_w32", tag="attn")
            nc.vector.memset(pw32[:, :], 0.0)
            for c in range(32):
                nc.tensor.matmul(pw32[:, c:c + 1],
                                 hsa_sb[:, 128 * c:128 * (c + 1)], hv[:, :],
                                 start=False, stop=False,
                                 skip_group_check=True)
            nc.scalar.activation(wtT_sb[:, :], pw32[:, :], AF.Tanh)
            ppro_ps = pp_acc.tile([D, 1], F32, name="ps_pro", tag="acc")
            for c in range(32):
                nc.tensor.matmul(ppro_ps[:, :], hsT_sb[:, D * c:D * (c + 1)],
                                 wtT_sb[:, c:c + 1],
                                 start=(c == 0), stop=(c == 31))
            nc.scalar.activation(ppro2[:, :], ppro_ps[:, :], AF.Copy,
                                 scale=1.0 / L)

            # ---------------- output MLP ------------------------------------
            # layer 0 takes compound (+bias) and protein as two accumulating
            # matmuls - no DMA round-trip to build the concatenated vector
            pm0 = pp_attn.tile([2 * D, 1], F32, name="ps_m0", tag="attn")
            nc.tensor.matmul(pm0[:, :],
                             cf_sb[0:D + 1, CF_WOC:CF_WOC + 2 * D],
                             cv32[:, :], start=True, stop=False)
            nc.tensor.matmul(pm0[:, :],
                             cf_sb[0:D, CF_WOP:CF_WOP + 2 * D],
                             ppro2[:, :], start=False, stop=True)
            nc.scalar.activation(catv[0:2 * D, 1:2], pm0[:, :], AF.Relu)
            for j in range(1, LO):
                pm = pp_attn.tile([2 * D, 1], F32, name=f"ps_m{j}", tag="attn")
                nc.tensor.matmul(pm[:, :], wo_sl(j), catv[:, j:j + 1],
                                 start=True, stop=True)
                nc.scalar.activation(catv[0:2 * D, j + 1:j + 2], pm[:, :],
                                     AF.Relu)
            pf = pp_attn.tile([2, 1], F32, name="ps_f", tag="attn")
            nc.tensor.matmul(pf[:, :], wi_sl, catv[:, LO:LO + 1],
                             start=True, stop=True)
            nc.scalar.activation(res_sb[:, :], pf[:, :], AF.Copy)
            nc.scalar.dma_start(out_d[:, :], res_sb[:, :])

    nc.compile()
    return nc


def _prep_inputs(inputs):
    fp = np.asarray(inputs["fingerprints"]).astype(np.int64)
    A = np.ascontiguousarray(np.asarray(inputs["adjacency"], dtype=np.float32))
    words = np.asarray(inputs["words"]).astype(np.int64)
    emb_fp = np.asarray(inputs["emb_fp"], dtype=np.float32)
    emb_word = np.asarray(inputs["emb_word"], dtype=np.float32)
    Wg_w = np.asarray(inputs["Wg_w"], dtype=np.float32)
    Wg_b = np.asarray(inputs["Wg_b"], dtype=np.float32)
    conv_w = np.asarray(inputs["conv_w"], dtype=np.float32)
    conv_b = np.asarray(inputs["conv_b"], dtype=np.float32)
    Wa_w = np.asarray(inputs["Wa_w"], dtype=np.float32)
    Wa_b = np.asarray(inputs["Wa_b"], dtype=np.float32)
    Wo_w = np.asarray(inputs["Wo_w"], dtype=np.float32)
    Wo_b = np.asarray(inputs["Wo_b"], dtype=np.float32)
    Wi_w = np.asarray(inputs["Wi_w"], dtype=np.float32)
    Wi_b = np.asarray(inputs["Wi_b"], dtype=np.float32)

    xs0 = emb_fp[fp]                                     # [N, D]
    ws0 = emb_word[words]                                # [L, D]

    A8 = A.astype(FP8_NP)
    shared = {}
    # S rows are (channel, tap)-ordered so the shift rebuild is one
    # overlapping-window DMA
    ws0_pad = np.zeros((D, LW), np.float32)
    ws0_pad[:, PAD:PAD + L] = ws0.T
    s0 = np.ones((KW * D + 1, L), np.float32)
    for c in range(D):
        for s in range(KW):
            s0[c * KW + s] = ws0_pad[c, s:s + L]
    shared["S0"] = s0.astype(BF16_NP)

    cf = np.zeros((NCORES * D, CF_W), np.float32)
    for i in range(LG):
        cf[0:D, CF_WG + D * i:CF_WG + D * (i + 1)] = Wg_w[i].T
        cf[D, CF_WG + D * i:CF_WG + D * (i + 1)] = Wg_b[i]
    for j in range(LO):
        cf[0:2 * D, CF_WO + 2 * D * j:CF_WO + 2 * D * (j + 1)] = Wo_w[j].T
        cf[2 * D, CF_WO + 2 * D * j:CF_WO + 2 * D * (j + 1)] = Wo_b[j]
    cf[0:2 * D, CF_WI:CF_WI + 2] = Wi_w.T
    cf[2 * D, CF_WI:CF_WI + 2] = Wi_b
    cf[0:D, CF_WOC:CF_WOC + 2 * D] = Wo_w[0].T[0:D]
    cf[D, CF_WOC:CF_WOC + 2 * D] = Wo_b[0]
    cf[0:D, CF_WOP:CF_WOP + 2 * D] = Wo_w[0].T[D:2 * D]
    cf[:, CF_SUM8:CF_SUM8 + D] = np.repeat(np.eye(D, dtype=np.float32),
                                           NCORES, axis=0)
    shared["CONSTF"] = cf

    cb = np.zeros((KW * D + 1, CB_W), np.float32)
    c_idx = np.arange(D)[:, None]
    d_idx = np.arange(D)[None, :]
    j = c_idx - d_idx + PAD                              # [c, d]
    valid = (j >= 0) & (j < KW)
    jc = np.clip(j, 0, KW - 1)
    for i in range(LC):
        w = conv_w[i, 0, 0]                              # [KW, KW] (s, j)
        for s in range(KW):
            # row (c, s): tap s of channel c
            cb[np.arange(D) * KW + s, CB_HST + D * i:CB_HST + D * (i + 1)] = \
                np.where(valid, w[s][jc], 0.0)
        cb[KW * D, CB_HST + D * i:CB_HST + D * (i + 1)] = conv_b[i]
    cb[0:D, CB_WA:CB_WA + D] = Wa_w.T
    cb[D, CB_WA:CB_WA + D] = Wa_b
    shared["CONSTB"] = cb.astype(BF16_NP)

    shared["ONESB"] = np.ones((1, L), BF16_NP)
    shared["ONESF"] = np.ones((1, NB), np.float32)

    in_maps = []
    for c in range(NCORES):
        blk = slice(NB * c, NB * (c + 1))
        m = dict(shared)
        at = np.ascontiguousarray(A8[blk].T)             # [N, NB] fp8
        m["AT"] = np.ascontiguousarray(
            at.reshape(NT, 128, NB).transpose(1, 0, 2).reshape(128, NT * NB))
        xs_c = np.ones((D + 1, NB), np.float32)
        xs_c[0:D] = xs0[blk].T
        m["XS0"] = xs_c
        in_maps.append(m)
    return in_maps


def run(inputs, trace=False, reps=1):
    from concourse.bass_utils import run_bass_kernel_spmd
    key = ("nc", reps)
    if key not in _CACHE:
        _CACHE[key] = _build_nc(reps)
    in_maps = _prep_inputs(inputs)
    res = run_bass_kernel_spmd(
        _CACHE[key], in_maps, core_ids=list(range(NCORES)), trace=trace)
    out = np.asarray(res.results[0]["OUT"], dtype=np.float32).reshape(1, 2)
    return out, res


def kernel(**inputs) -> np.ndarray:
    out, _ = run(inputs, trace=False)
    return out
